# revision 1
# baseline (speedup 1.0000x reference)
"""AttentionReadout Trainium2 kernel (8-core data-parallel over graphs).

Reference computation (per graph of 64 nodes, D=512, H=8 heads, hd=64):
    qkv = x @ in_proj_w.T + in_proj_b ; q,k,v = split(qkv)
    attn = softmax(q k^T / sqrt(hd)) v          (per head)
    attn_out = attn @ out_proj_w.T + out_proj_b
    gates = sigmoid(attn_out @ gate_w.T + gate_b)
    out[g] = sum_n attn_out[n] * gates[n]

Key algebraic restructure vs the naive chain: with weff = out_proj_w.T @
gate_w and gb_eff = gate_b + out_proj_b @ gate_w,
    gates  = sigmoid(ctx @ weff + gb_eff)           (no attn_out needed)
    out[g] = (sum_n gates_n * ctx_n) @ out_proj_w.T + (sum_n gates_n) * bo
so the out-projection runs ONCE per core on [128 graphs, D], not per node.

Layout strategy (per core: 128 graphs = 8192 nodes, superblock = 512 nodes):
  - x arrives PRE-TRANSPOSED from the host ([128, DC, rows] bf16): no
    on-device transposes, plain contiguous DMA loads only.
  - Q^T,K^T projected in [e, n] orientation one superblock AHEAD; odd
    heads' rows realigned to partition base 0 via a direct SBUF->SBUF
    DMA on the scalar ring (hidden under the previous block phase).
  - scores S^T[m, n] per (128-node block, head) into a [128, 8, 128] psum;
    exp on ScalarE reads the two valid 64x64 diagonal quadrant sets in two
    8-head strided instructions; attn off-diagonal quadrants are stale
    zeros (pool buffers pre-zeroed once).
  - ctx natural [n, e] per head via stationary attn / moving
    [v | vg | ones] (N=66: ctx + gate numerator + rowsum in one shot),
    deferred 2 pipeline steps; normalized by 1/rowsum on VectorE.
  - gate = 0.5*tanh(0.5(sum_h vgnum_h/rs_h) + 0.5 gb) + 0.5 (tiny ops;
    the v.weff column is precomputed at V-projection time).
  - readout: per block one matmul, stationary G[128, 32] (gate columns,
    zeros elsewhere), moving ctxn [128, 512], accumulated across 16 blocks
    per 4-superblock group into an exclusive psum bank -> r[g, e] for all
    128 graphs in one bank.
  - tail: r -> (PE transpose) -> r^T -> 4 matmuls vs wo -> out.
"""

import numpy as np
import ml_dtypes

import concourse.bass as bass
import concourse.mybir as mybir
import concourse.tile as tile
from concourse import bacc
from concourse.bass_utils import run_bass_kernel_spmd
from concourse.masks import make_identity

F32 = mybir.dt.float32
BF16 = mybir.dt.bfloat16

N_CORES = 8
D = 512
H = 8
HD = 64
NPG = 64            # nodes per graph
TOTAL = 65536
ROWS = TOTAL // N_CORES      # 8192 nodes per core
GC = ROWS // NPG             # 128 graphs per core
BLK = 128                    # nodes per block (2 graphs)
SBN = 512                    # nodes per superblock (4 blocks, 8 graphs)
NSB = ROWS // SBN            # 16 superblocks
NBLK = SBN // BLK            # 4 blocks per superblock
DC = D // 128                # 4 d-chunks
GRP = 4                      # superblocks per readout group (32 graphs)

# module-level switch used by test.py; harness default is no tracing
TRACE = False

try:
    import jax as _jax
    _jax.config.update("jax_compilation_cache_dir", "/tmp/jax_neff_cache")
    _jax.config.update("jax_persistent_cache_min_compile_time_secs", 10)
    _jax.config.update("jax_persistent_cache_min_entry_size_bytes", 0)
except Exception:
    pass


def _build(has_bqk, has_bv, has_bo, has_gb, gb_eff=0.0, rows=ROWS, variant=()):
    variant = set(variant)
    stage = 9
    for _v in variant:
        if _v.startswith("s") and _v[1:].isdigit():
            stage = int(_v[1:])
    nsb = rows // SBN
    gc = rows // NPG
    nc = bacc.Bacc(None, target_bir_lowering=False, debug=False)

    xbf = nc.dram_tensor("xbf", [128, DC, rows], BF16, kind="ExternalInput")
    wqk = nc.dram_tensor("wqk", [128, DC, 2 * D], BF16, kind="ExternalInput")
    wv = nc.dram_tensor("wv", [128, DC, D], BF16, kind="ExternalInput")
    wo = nc.dram_tensor("wo", [128, DC, D], BF16, kind="ExternalInput")
    weff = nc.dram_tensor("weff", [1, D], F32, kind="ExternalInput")
    qkoh = nc.dram_tensor("qkoh", [2, 64, 8, SBN], BF16, kind="Internal")
    if has_bqk:
        bqk = nc.dram_tensor("bqk", [128, 8], F32, kind="ExternalInput")
    if has_bv:
        bv = nc.dram_tensor("bv", [1, D], F32, kind="ExternalInput")
    if has_bo:
        bo = nc.dram_tensor("bo", [1, D], F32, kind="ExternalInput")
    out = nc.dram_tensor("out", [gc, D], F32, kind="ExternalOutput")

    from contextlib import ExitStack
    with tile.TileContext(nc) as tc, ExitStack() as st:
        consts = st.enter_context(tc.tile_pool(name="consts", bufs=1))
        p_xt = st.enter_context(tc.tile_pool(name="p_xt", bufs=3))
        p_qkt = st.enter_context(tc.tile_pool(name="p_qkt", bufs=2))
        p_qko = st.enter_context(tc.tile_pool(name="p_qko", bufs=2))
        p_vtx = st.enter_context(tc.tile_pool(name="p_vtx", bufs=4))
        p_attn = st.enter_context(tc.tile_pool(name="p_attn", bufs=3))
        p_ctxn = st.enter_context(tc.tile_pool(name="p_ctxn", bufs=3))
        p_small = st.enter_context(tc.tile_pool(name="p_small", bufs=4))
        p_G = st.enter_context(tc.tile_pool(name="p_G", bufs=16))
        p_out = st.enter_context(tc.tile_pool(name="p_out", bufs=1))
        ps_big = st.enter_context(tc.tile_pool(name="ps_big", bufs=2, space="PSUM"))
        ps_s = st.enter_context(tc.tile_pool(name="ps_s", bufs=3, space="PSUM"))
        ps_c = st.enter_context(tc.tile_pool(name="ps_c", bufs=2, space="PSUM"))
        ps_r = st.enter_context(tc.tile_pool(name="ps_r", bufs=1, space="PSUM"))

        # ---- weights / constants; order matters for startup overlap ----
        # superblock 0's X^T first (x arrives pre-transposed from the
        # host), then wqk in four chunks so the first QK ec-groups start
        # as soon as their slice lands
        xt0 = p_xt.tile([128, DC, SBN], BF16, tag="xt", name="xt0")
        nc.sync.dma_start(xt0[:, :, :], xbf[:, :, 0:SBN])
        wqk_sb = consts.tile([128, DC, 2 * D], BF16, tag="wqk")
        for q in range(4):
            nc.sync.dma_start(
                wqk_sb[:, :, q * D // 2:(q + 1) * D // 2],
                wqk[:, :, q * D // 2:(q + 1) * D // 2])

        wv_sb = consts.tile([128, DC, D], BF16, tag="wv")
        nc.sync.dma_start(wv_sb[:], wv[:, :, :])

        weff_row = consts.tile([1, D], F32, tag="weff_row")
        nc.sync.dma_start(weff_row[:], weff[:, :])
        weff_f32 = consts.tile([128, D], F32, tag="weff_f32")
        nc.gpsimd.partition_broadcast(weff_f32[:], weff_row[:])
        weff_bc = consts.tile([128, D], BF16, tag="weff_bc")
        nc.vector.tensor_copy(weff_bc[:], weff_f32[:])

        if has_bqk:
            bqk_sb = consts.tile([128, 8], F32, tag="bqk")
            nc.sync.dma_start(bqk_sb[:], bqk[:, :])
        if has_bv:
            bv_row = consts.tile([1, D], F32, tag="bv_row")
            nc.sync.dma_start(bv_row[:], bv[:, :])
            bv_full = consts.tile([128, D], F32, tag="bv_full")
            nc.gpsimd.partition_broadcast(bv_full[:], bv_row[:])
        if has_bo:
            bo_row = consts.tile([1, D], F32, tag="bo_row")
            nc.sync.dma_start(bo_row[:], bo[:, :])
            bo_full = consts.tile([128, D], F32, tag="bo_full")
            nc.gpsimd.partition_broadcast(bo_full[:], bo_row[:])
            s_row = consts.tile([1, gc], F32, tag="s_row")

        # readout accumulator: one psum bank, exclusively owned
        rps = ps_r.tile([128, D], F32, tag="r")

        # pre-zero attn pool buffers' off-diagonal quadrants (they are
        # never dirtied: exp writes only the diagonal quadrants)
        for _ in range(3):
            az = p_attn.tile([128, H, BLK], BF16, tag="attn", name="az")
            nc.gpsimd.memset(az[0:64, :, 64:128], 0.0)
            nc.gpsimd.memset(az[64:128, :, 0:64], 0.0)
        # pre-set vtx ones column (layout: [v 0:64 | vg 64 | ones 65])
        for _ in range(4):
            vz = p_vtx.tile([128, H, HD + 2], BF16, tag="vtx", name="vz")
            nc.vector.memset(vz[:, :, HD + 1:HD + 2], 1.0)
        # pre-zero G buffers
        for _ in range(16):
            gz_ = p_G.tile([128, 32], BF16, tag="G", name="gzb")
            nc.gpsimd.memset(gz_[:], 0.0)

        # tail-only constants, emitted last so they never gate the loop
        ident_f32 = consts.tile([128, 128], F32, tag="ident_f32")
        make_identity(nc, ident_f32[:])
        wo_sb = consts.tile([128, DC, D], BF16, tag="wo")
        nc.sync.dma_start(wo_sb[:], wo[:, :, :])

        # ---- main loop ----
        # ctx / readout matmuls are deferred (software pipelining) so the
        # exp -> ctx and gate -> readout latencies hide under tensor work
        pending_ctx = []
        pending_ro = []

        def _gate(sb, b, ctxn, zp):
            # gate: z = sum_h zp[h] ; gate = 0.5*tanh(0.5 z + 0.5 gb)+0.5
            gzs = p_small.tile([128, 1], F32, tag="gzs")
            cflat = ctxn[:].rearrange("p h c -> p (h c)")
            gt = p_small.tile([128, 1], F32, tag="gt")
            nc.vector.tensor_reduce(
                gzs[:], zp[:], mybir.AxisListType.X, mybir.AluOpType.add)
            nc.scalar.activation(
                gt[:], gzs[:], mybir.ActivationFunctionType.Tanh,
                bias=(0.5 * gb_eff) if has_gb else 0.0, scale=0.5)

            G = p_G.tile([128, 32], BF16, tag="G")
            c0 = 8 * (sb % GRP) + 2 * b
            nc.gpsimd.tensor_scalar(
                G[0:64, c0:c0 + 1], gt[0:64, :], 0.5, 0.5,
                mybir.AluOpType.mult, mybir.AluOpType.add)
            nc.gpsimd.tensor_scalar(
                G[64:128, c0 + 1:c0 + 2], gt[64:128, :], 0.5, 0.5,
                mybir.AluOpType.mult, mybir.AluOpType.add)

            if has_bo:
                g0 = 8 * sb + 2 * b
                nc.gpsimd.tensor_reduce(
                    s_row[0:1, g0:g0 + 1], gt[0:64, :],
                    mybir.AxisListType.C, mybir.AluOpType.add)
                nc.gpsimd.tensor_reduce(
                    s_row[0:1, g0 + 1:g0 + 2], gt[64:128, :],
                    mybir.AxisListType.C, mybir.AluOpType.add)

            if stage <= 7:
                return
            # readout: accumulate r[g, e] for this 4-superblock group
            k = 0 if "ro0" in variant else sb // GRP
            first = (sb % GRP == 0) and (b == 0)
            last = (sb % GRP == GRP - 1) and (b == NBLK - 1)
            if "noacc" in variant:
                first = last = True

            def _ro(G=G, cflat=cflat, k=k, first=first, last=last):
                nc.tensor.matmul(
                    rps[32 * k:32 * k + 32, :], G[:], cflat,
                    start=first, stop=last,
                    tile_position=(0, 32 * k))
            pending_ro.append(_ro)
            flush(pending_ro, keep=2)


        def flush(q, keep=0):
            while len(q) > keep:
                q.pop(0)()

        def emit_xt(s):
            t = p_xt.tile([128, DC, SBN], BF16, tag="xt", name="xt")
            nc.sync.dma_start(t[:, :, :], xbf[:, :, s * SBN:(s + 1) * SBN])
            return t

        def emit_qk_ec(s, xt, qkt, ec):
            ps = ps_big.tile([128, SBN], F32, tag="big", name="psq")
            for dc in range(DC):
                nc.tensor.matmul(
                    ps[:],
                    wqk_sb[:, dc, ec * 128:(ec + 1) * 128],
                    xt[:, dc, :],
                    start=(dc == 0), stop=(dc == DC - 1))
            if has_bqk:
                if ec % 2 == 0:
                    nc.vector.tensor_scalar_add(
                        qkt[:, ec, :], ps[:], bqk_sb[:, ec:ec + 1])
                else:
                    nc.scalar.activation(
                        qkt[:, ec, :], ps[:],
                        mybir.ActivationFunctionType.Identity,
                        bias=bqk_sb[:, ec:ec + 1])
            else:
                if ec % 2 == 0:
                    nc.vector.tensor_copy(qkt[:, ec, :], ps[:])
                else:
                    nc.scalar.copy(qkt[:, ec, :], ps[:])

        def emit_bounce(s, qkt):
            # realign odd heads' rows (partitions 64:127) to base 0 with a
            # direct SBUF->SBUF DMA on the scalar ring (safe: no
            # dma-transposes anywhere in this program)
            qko = p_qko.tile([64, 8, SBN], BF16, tag="qko", name="qko")
            nc.scalar.dma_start(qko[:, :, :], qkt[64:128, :, :])
            return qko

        # prologue: xt prefetch 2 deep, QK one superblock ahead
        xts = {0: xt0}
        if nsb > 1:
            xts[1] = emit_xt(1)
        qks = {}
        if stage >= 2:
            qkt0 = p_qkt.tile([128, 8, SBN], BF16, tag="qkt", name="qkt0")
            for ec in range(8):
                emit_qk_ec(0, xts[0], qkt0, ec)
            qks[0] = (qkt0, emit_bounce(0, xts[0] and qkt0))

        for sb in range(nsb):
            if sb + 2 < nsb:
                xts[sb + 2] = emit_xt(sb + 2)
            if stage <= 2:
                continue

            xt = xts.pop(sb)
            # queue of "big" matmul groups (V proj for this superblock,
            # QK proj for the next) to interleave into the block phase so
            # the PE activity monitor never re-throttles the clock
            bigq = []
            vts = [None] * NBLK

            def _vgrp(b, xt=xt):
                psv = ps_big.tile([128, SBN], F32, tag="big", name="psv")
                for dc in range(DC):
                    nc.tensor.matmul(
                        psv[:],
                        xt[:, dc, b * 128:(b + 1) * 128],
                        wv_sb[:, dc, :],
                        start=(dc == 0), stop=(dc == DC - 1))
                vtx = p_vtx.tile([128, H, HD + 2], BF16, tag="vtx")
                pv = psv[:].rearrange("p (h c) -> p h c", h=H)
                if has_bv:
                    nc.vector.tensor_tensor(
                        vtx[:, :, 0:HD], pv,
                        bv_full[:].rearrange("p (h c) -> p h c", h=H),
                        mybir.AluOpType.add)
                else:
                    nc.vector.tensor_copy(vtx[:, :, 0:HD], pv)
                # vg[n, h] = v[n, h, :] . weff[h, :]  (gate numerator seed),
                # computed here so the per-block gate chain is tiny
                scr = p_small.tile([128, H, HD], BF16, tag="scr")
                nc.gpsimd.tensor_tensor(
                    scr[:], vtx[:, :, 0:HD],
                    weff_bc[:].rearrange("p (h c) -> p h c", h=H),
                    mybir.AluOpType.mult)
                vgt = p_small.tile([128, H], F32, tag="vgt")
                nc.vector.tensor_reduce(
                    vgt[:], scr[:], mybir.AxisListType.X, mybir.AluOpType.add)
                nc.vector.tensor_copy(vtx[:, :, HD], vgt[:])
                vts[b] = vtx

            for b in range(NBLK):
                bigq.append(lambda b=b: _vgrp(b))
            if sb + 1 < nsb:
                xt_n = xts[sb + 1]
                qkt_n = p_qkt.tile([128, 8, SBN], BF16, tag="qkt", name="qktn")
                for ec in range(8):
                    bigq.append(
                        lambda ec=ec, xt_n=xt_n, qkt_n=qkt_n:
                        emit_qk_ec(sb + 1, xt_n, qkt_n, ec))

                def _fin_qk(s=sb + 1, qkt_n=qkt_n):
                    qks[s] = (qkt_n, emit_bounce(s, qkt_n))
            else:
                _fin_qk = None

            # all big matmul groups run contiguously before the block
            # phase — back-to-back N=512 matmuls pipeline best
            while bigq:
                bigq.pop(0)()
            if _fin_qk is not None:
                _fin_qk()
                _fin_qk = None

            if stage <= 3:
                continue
            qkt, qko = qks.pop(sb)
            # per block: attention + gates + readout, software-pipelined
            for b in range(NBLK):
                n0 = b * BLK
                attn = p_attn.tile([128, H, BLK], BF16, tag="attn")
                ctxn = p_ctxn.tile([128, H, HD], BF16, tag="ctxn")
                zp = p_small.tile([128, H], F32, tag="zp")

                for hh in range(2):
                    pss = ps_s.tile([128, 4, BLK], F32, tag="s")
                    for j in range(4):
                        h = hh * 4 + j
                        src = qkt if h % 2 == 0 else qko
                        nc.tensor.matmul(
                            pss[:, j, :],
                            src[0:64, 4 + h // 2, n0:n0 + BLK],
                            src[0:64, h // 2, n0:n0 + BLK],
                            start=True, stop=True)
                    if stage > 4:
                        h0 = hh * 4
                        nc.scalar.activation(
                            attn[0:64, h0:h0 + 4, 0:64],
                            pss[0:64, :, 0:64],
                            mybir.ActivationFunctionType.Exp, scale=0.125)
                        nc.scalar.activation(
                            attn[64:128, h0:h0 + 4, 64:128],
                            pss[64:128, :, 64:128],
                            mybir.ActivationFunctionType.Exp, scale=0.125)


                    if stage <= 5:
                        continue

                    def _ctx(sb=sb, b=b, hh=hh, attn=attn, ctxn=ctxn,
                             zp=zp, vtx=vts[b]):
                        psc = ps_c.tile([128, 4, 128], F32, tag="c")
                        for j in range(4):
                            h = hh * 4 + j
                            nc.tensor.matmul(
                                psc[:, j, 0:HD + 2],
                                attn[:, h, :],
                                vtx[:, h, :],
                                start=True, stop=True)
                        rr = p_small.tile([128, 4], F32, tag="rr")
                        nc.vector.reciprocal(rr[:], psc[:, :, HD + 1])
                        nc.vector.tensor_tensor(
                            ctxn[:, hh * 4:hh * 4 + 4, :],
                            psc[:, :, 0:HD],
                            rr[:, :, None].to_broadcast((128, 4, HD)),
                            mybir.AluOpType.mult)
                        nc.vector.tensor_tensor(
                            zp[:, hh * 4:hh * 4 + 4], psc[:, :, HD], rr[:],
                            mybir.AluOpType.mult)
                        if hh == 1 and stage >= 7:
                            _gate(sb, b, ctxn, zp)

                    pending_ctx.append(_ctx)
                    flush(pending_ctx, keep=2)

            # anything left (e.g. last superblock with no next QK)
            while bigq:
                bigq.pop(0)()
            if _fin_qk is not None:
                _fin_qk()

        # ---- tail: r -> r^T -> out projection ----
        flush(pending_ctx, keep=0)
        flush(pending_ro, keep=0)
        rsb = p_out.tile([128, D], F32, tag="rsb")
        if stage <= 8:
            nc.vector.memset(rsb[:], 0.0)
        else:
            nc.vector.tensor_copy(rsb[:], rps[:])
        ptt = ps_c.tile([128, DC, 128], F32, tag="c", name="ptt")
        for dc in range(DC):
            nc.tensor.transpose(
                ptt[:, dc, :], rsb[:, dc * 128:(dc + 1) * 128], ident_f32[:])
        rt = p_out.tile([128, DC, 128], BF16, tag="rt")
        nc.scalar.copy(rt[:], ptt[:])
        pso = ps_big.tile([128, D], F32, tag="big", name="pso")
        for dc in range(DC):
            nc.tensor.matmul(
                pso[:], rt[:, dc, :], wo_sb[:, dc, :],
                start=(dc == 0), stop=(dc == DC - 1))
        out_sb = p_out.tile([128, D], F32, tag="osb")
        if has_bo:
            # out += (sum_n gate_n) * bo : transpose s_row to [gc, 1]
            pst = ps_c.tile([128, 4, 128], F32, tag="c", name="pst")
            nc.tensor.transpose(pst[0:gc, 0, 0:1], s_row[:, :], ident_f32[:])
            s_col = p_out.tile([128, 1], F32, tag="s_col")
            nc.vector.tensor_copy(s_col[0:gc, :], pst[0:gc, 0, 0:1])
            sbo = p_out.tile([128, D], F32, tag="sbo")
            nc.vector.tensor_scalar_mul(
                sbo[:], bo_full[:], s_col[:, 0:1])
            nc.vector.tensor_tensor(
                out_sb[:], pso[:], sbo[:], mybir.AluOpType.add)
        else:
            nc.vector.tensor_copy(out_sb[:], pso[:])
        nc.sync.dma_start(out[:, :], out_sb[0:gc, :])

    import time as _time
    _t = _time.time()
    nc.compile()
    print(f"[kernel] bacc compile: {_time.time()-_t:.1f}s", flush=True)
    return nc


def kernel(x, batch, in_proj_w, in_proj_b, out_proj_w, out_proj_b,
           gate_w, gate_b):
    x = np.ascontiguousarray(np.asarray(x, dtype=np.float32))
    in_proj_w = np.asarray(in_proj_w, dtype=np.float32)
    in_proj_b = np.asarray(in_proj_b, dtype=np.float32)
    out_proj_w = np.asarray(out_proj_w, dtype=np.float32)
    out_proj_b = np.asarray(out_proj_b, dtype=np.float32)
    gate_w = np.asarray(gate_w, dtype=np.float32)
    gate_b = np.asarray(gate_b, dtype=np.float32)

    # host-side weight prep
    wqkT = in_proj_w[:2 * D].T                              # [512, 1024]
    wqk_h = np.ascontiguousarray(
        wqkT.reshape(DC, 128, 2 * D).transpose(1, 0, 2)).astype(ml_dtypes.bfloat16)
    wvT = in_proj_w[2 * D:].T                               # [512, 512]
    wv_h = np.ascontiguousarray(
        wvT.reshape(DC, 128, D).transpose(1, 0, 2)).astype(ml_dtypes.bfloat16)
    woT = out_proj_w.T                                      # [512, 512]
    wo_h = np.ascontiguousarray(
        woT.reshape(DC, 128, D).transpose(1, 0, 2)).astype(ml_dtypes.bfloat16)
    weff_h = (out_proj_w.T @ gate_w[0]).astype(np.float32).reshape(1, D)

    bqk_np = in_proj_b[:2 * D]
    bv_np = in_proj_b[2 * D:]
    gb_eff = float(gate_b[0] + out_proj_b @ gate_w[0])
    has_bqk = bool(np.any(bqk_np))
    has_bv = bool(np.any(bv_np))
    has_bo = bool(np.any(out_proj_b))
    has_gb = gb_eff != 0.0

    import time as _time
    _t = _time.time()
    nc = _build(has_bqk, has_bv, has_bo, has_gb, gb_eff=gb_eff)
    print(f"[kernel] build total: {_time.time()-_t:.1f}s", flush=True)

    in_maps = []
    for c in range(N_CORES):
        xc = x[c * ROWS:(c + 1) * ROWS].astype(ml_dtypes.bfloat16)
        xct = np.ascontiguousarray(
            xc.T.reshape(DC, 128, ROWS).transpose(1, 0, 2))
        m = {
            "xbf": xct,
            "wqk": wqk_h, "wv": wv_h, "wo": wo_h, "weff": weff_h,
        }
        if has_bqk:
            m["bqk"] = np.ascontiguousarray(
                bqk_np.reshape(8, 128).T).astype(np.float32)
        if has_bv:
            m["bv"] = bv_np.reshape(1, D).astype(np.float32)
        if has_bo:
            m["bo"] = out_proj_b.reshape(1, D).astype(np.float32)
        in_maps.append(m)

    kernel.last_nc = nc
    kernel.last_in_maps = in_maps
    kernel.last_flags = (has_bqk, has_bv, has_bo, has_gb)

    res = run_bass_kernel_spmd(
        nc, in_maps, core_ids=list(range(N_CORES)), trace=TRACE)
    if TRACE:
        kernel.last_exec_time_ns = res.exec_time_ns
        kernel.last_results = res

    return np.concatenate([r["out"] for r in res.results], axis=0)


kernel.last_exec_time_ns = None
kernel.last_results = None
kernel.last_nc = None
kernel.last_in_maps = None

